# revision 9
# baseline (speedup 1.0000x reference)
"""Trainium2 Bass kernel for windowed embedding lookup (nn_AttentionLayer).

Computation:
  out[b,s,e] = sum_k w[k,e] * data[snip_b, clip(inputs[b,s]+k-5, 0, 165), 0, e]

Strategy (data-parallel over batch, 2 batches per core on 8 cores):
  1. Host stages the table as bf16 with the clip-padding baked in
     ([100*128, 6*176]: per snippet, e-major chunks of 176 padded
     positions), so each batch's slice is ONE contiguous HWDGE dynamic
     DMA (snippet id in a sync/scalar-engine register via values_load).
  2. Diagonal weight matrices diag(w[k, e-chunk]) are built on-device
     from a tiny [128, 66] staged weight tile via affine_select
     (broadcast-read + off-diagonal fill 0) on DVE/GPSIMD.
  3. 11-tap clip-padded convolution C[p,e] = sum_k w[k,e]*T[p+k-5,e]
     on TensorE as PSUM-accumulated matmuls (lhsT = shifted T window,
     rhs = diag), 2 row-blocks x 6 chunks x 11 taps per batch.
  4. Row gather out[s] = C[inputs[s]] as one-hot matmuls (iota +
     is_equal one-hots, 2 row-blocks accumulated in PSUM).
  5. PSUM drained to SBUF on DVE/ACT/GPSIMD round-robin; per-tile
     output DMAs issued alternately from sync/scalar HWDGE queues.
  A few warm-up matmuls run during the DMA preamble so the PE p-state
  is ramped when the real work arrives.
"""

import sys

for _p in ("/opt/trn_rl_repo",):
    if _p not in sys.path:
        sys.path.insert(0, _p)

import numpy as np

N_CORES = 8
B = 16
BPC = B // N_CORES  # batches per core
S = 1126
E = 768
EC = 6  # number of 128-wide e chunks
P = 166  # table positions
PPAD = 176  # padded positions (5 on each side)
W = 11
NSNIP = 100
MTILES = (S + 127) // 128  # 9
WARM_MMS = 4

_cache = {}


def _build():
    import concourse.bass as bass
    import concourse.mybir as mybir
    import concourse.tile as tile
    from concourse import bacc

    f32 = mybir.dt.float32
    bf16 = mybir.dt.bfloat16
    i32 = mybir.dt.int32
    AOT = mybir.AluOpType
    ET = mybir.EngineType

    nc = bacc.Bacc()

    snips_d = nc.declare_dram_parameter("snips", [1, BPC], i32, isOutput=False)
    inps_d = nc.declare_dram_parameter(
        "inps", [1, BPC * S], bf16, isOutput=False
    )
    # row (snip*128 + i) holds [c*176 + j] -> data[snip, clip(j-5), 0, c*128+i]
    dataT2p = nc.declare_dram_parameter(
        "dataT2p", [NSNIP * 128, EC * PPAD], bf16, isOutput=False
    )
    # w2[i, c*11+k] = w[k, c*128+i]
    w2_d = nc.declare_dram_parameter("w2", [128, EC * W], bf16, isOutput=False)
    out = nc.declare_dram_parameter("out", [BPC * S, E], f32, isOutput=True)

    with tile.TileContext(nc) as tc:
        with (
            tc.tile_pool(name="const", bufs=1) as constp,
            tc.tile_pool(name="work", bufs=1) as workp,
            tc.tile_pool(name="ob", bufs=6) as obp,
            tc.tile_pool(name="psA", bufs=2, space="PSUM") as psA,
            tc.tile_pool(name="psB", bufs=2, space="PSUM") as psB,
        ):
            # ---------- tiny constants ----------
            ones1 = constp.tile([1, 128], bf16)
            nc.vector.memset(ones1[:], 1.0)
            warm = constp.tile([128, 512], bf16)
            nc.vector.memset(warm[:], 0.001)

            iota_i = constp.tile([128, 1], i32)
            nc.gpsimd.iota(iota_i[:], [[1, 1]], base=0, channel_multiplier=1)
            iota_f = constp.tile([128, 2], f32)
            nc.vector.tensor_copy(iota_f[:, 0:1], iota_i[:])
            nc.vector.tensor_scalar_add(iota_f[:, 1:2], iota_f[:, 0:1], 128.0)

            # ---------- input DMAs (issue ASAP, spread across queues) ----
            snipt = workp.tile([1, BPC], i32, tag="snipt")
            nc.sync.dma_start(out=snipt[:], in_=snips_d[:])
            inprt = workp.tile([1, BPC * S], bf16, tag="inprt")
            nc.sync.dma_start(out=inprt[:], in_=inps_d[:])
            w2b = constp.tile([128, EC * W], bf16)
            nc.scalar.dma_start(out=w2b[:], in_=w2_d[:])

            snip_v = [
                nc.values_load(
                    snipt[0:1, 0:1],
                    engines=[ET.SP],
                    min_val=0,
                    max_val=NSNIP - 1,
                    skip_runtime_bounds_check=True,
                ),
                nc.values_load(
                    snipt[0:1, 1:2],
                    engines=[ET.Activation],
                    min_val=0,
                    max_val=NSNIP - 1,
                    skip_runtime_bounds_check=True,
                ),
            ]
            t2 = []
            for b, eng in ((0, nc.sync), (1, nc.scalar)):
                t2b = workp.tile([128, EC, PPAD], bf16, tag=f"t2_{b}")
                eng.dma_start(
                    out=t2b[:, :, :],
                    in_=dataT2p[bass.ts(snip_v[b], 128), :].rearrange(
                        "p (c j) -> p c j", j=PPAD
                    ),
                )
                t2.append(t2b)

            # ---------- diag weights built on device ------------------
            # diagb[i, c*11+k, j] = w2[i, c*11+k] if i == j else 0
            from concourse.masks import make_identity

            identb = constp.tile([128, 128], bf16)
            make_identity(nc, identb[:, :])
            ident_bc = identb[:, :].rearrange(
                "p (u j) -> p u j", u=1
            ).to_broadcast([128, W, 128])

            diagb = constp.tile([128, EC * W, 128], bf16)
            w2_bcast = w2b[:, :].rearrange("p (c u) -> p c u", u=1)
            for c in range(EC):
                wslice = w2_bcast[:, c * W : (c + 1) * W, :].to_broadcast(
                    [128, W, 128]
                )
                if c % 2 == 0:
                    nc.vector.tensor_tensor(
                        out=diagb[:, c * W : (c + 1) * W, :],
                        in0=ident_bc,
                        in1=wslice,
                        op=AOT.mult,
                    )
                else:
                    nc.gpsimd.affine_select(
                        out=diagb[:, c * W : (c + 1) * W, :],
                        in_=wslice,
                        compare_op=AOT.is_equal,
                        fill=0.0,
                        base=0,
                        # keep where (i - j) == 0
                        pattern=[[0, W], [-1, 128]],
                        channel_multiplier=1,
                    )

            # ---------- PE warm-up (ramp the p-state) ------------------
            warm_ps = psB.tile([128, E], f32, tag="go")
            for wi in range(WARM_MMS):
                nc.tensor.matmul(
                    out=warm_ps[:, 0:512],
                    lhsT=warm[:, 0:128],
                    rhs=warm[:, 0:512],
                    start=(wi == 0),
                    stop=(wi == WARM_MMS - 1),
                )
            warm_close = constp.tile([128, 1], f32)
            nc.vector.tensor_copy(warm_close[:], warm_ps[:, 0:1])

            # ---------- input broadcast + one-hots ---------------------
            # inpb[b][p, s] = inputs[b, s] replicated over 128 partitions
            inpb = []
            chunks = [(0, 512), (512, 512), (1024, S - 1024)]
            for b in range(BPC):
                ib = workp.tile([128, S], bf16, tag=f"inpb{b}")
                for ci, (n0, nw) in enumerate(chunks):
                    ps_in = psB.tile([128, E], f32, tag="go")
                    nc.tensor.matmul(
                        out=ps_in[:, :nw],
                        lhsT=ones1[:, :],
                        rhs=inprt[0:1, b * S + n0 : b * S + n0 + nw],
                        start=True,
                        stop=True,
                    )
                    if ci % 2 == 0:
                        nc.vector.tensor_copy(ib[:, n0 : n0 + nw], ps_in[:, :nw])
                    else:
                        nc.scalar.copy(ib[:, n0 : n0 + nw], ps_in[:, :nw])
                inpb.append(ib)

            oh = []
            for b in range(BPC):
                oh0 = workp.tile([128, S], bf16, tag=f"oh0_{b}")
                oh1 = workp.tile([128, S], bf16, tag=f"oh1_{b}")
                e0 = nc.vector if b == 0 else nc.gpsimd
                e1 = nc.gpsimd if b == 0 else nc.vector
                e0.tensor_scalar(
                    oh0[:], inpb[b][:], iota_f[:, 0:1], None, AOT.is_equal
                )
                e1.tensor_scalar(
                    oh1[:], inpb[b][:], iota_f[:, 1:2], None, AOT.is_equal
                )
                oh.append((oh0, oh1))

            # ---------- conv on PE: C = sum_k diag(w_k) shifted -------
            # GPSIMD cannot touch PSUM: drains alternate DVE / ACT only.
            def drain(idx, dst, src):
                if idx % 2 == 0:
                    nc.vector.tensor_copy(dst, src)
                else:
                    nc.scalar.copy(dst, src)
            ccat = []
            for b in range(BPC):
                for blk in range(2):
                    mw = 128 if blk == 0 else P - 128
                    psc = psA.tile([128, E], f32, tag="cv")
                    for c in range(EC):
                        for k in range(W):
                            nc.tensor.matmul(
                                out=psc[:mw, c * 128 : (c + 1) * 128],
                                lhsT=t2[b][:, c, blk * 128 + k : blk * 128 + k + mw],
                                rhs=diagb[:, c * W + k, :],
                                start=(k == 0),
                                stop=(k == W - 1),
                            )
                    cc = workp.tile([128, E], bf16, tag=f"cc{b}_{blk}")
                    drain(b * 2 + blk, cc[:mw, :], psc[:mw, :])
                    ccat.append(cc)

            # ---------- gather + drain + store -------------------------
            # Output DMAs grouped: m-tiles (0,1,2) (3,4,5) (6,7) (8) per
            # batch become 4 strided DMAs (fewer, bigger HWDGE issues).
            groups = [(0, 3), (3, 3), (6, 2), (8, 1)]
            for b in range(BPC):
                cc0, cc1 = ccat[b * 2], ccat[b * 2 + 1]
                oh0, oh1 = oh[b]
                for gi, (m0, gn) in enumerate(groups):
                    ob = obp.tile([128, 3, E], f32, tag="ob")
                    for g in range(gn):
                        m = m0 + g
                        mw = min(128, S - m * 128)
                        pso = psB.tile([128, E], f32, tag="go")
                        for ohx, ccx, st in ((oh0, cc0, True), (oh1, cc1, False)):
                            for n0, nw in ((0, 512), (512, 256)):
                                nc.tensor.matmul(
                                    out=pso[:mw, n0 : n0 + nw],
                                    lhsT=ohx[:, m * 128 : m * 128 + mw],
                                    rhs=ccx[:, n0 : n0 + nw],
                                    start=st,
                                    stop=not st,
                                )
                        drain(b * MTILES + m, ob[:mw, g, :], pso[:mw, :])
                    r0 = b * S + m0 * 128
                    nrows = min(gn * 128, S - m0 * 128)
                    dma_eng = nc.sync if (b * 4 + gi) % 2 == 0 else nc.scalar
                    if nrows == gn * 128:
                        dma_eng.dma_start(
                            out=out[r0 : r0 + nrows, :].rearrange(
                                "(g p) e -> p g e", p=128
                            ),
                            in_=ob[:, 0:gn, :],
                        )
                    else:
                        dma_eng.dma_start(
                            out=out[r0 : r0 + nrows, :],
                            in_=ob[:nrows, 0, :],
                        )

    nc.finalize()
    return nc


def _get_nc():
    if "nc" not in _cache:
        _cache["nc"] = _build()
    return _cache["nc"]


def _prep_shared(data, w):
    # layout-only host staging (transpose/reshape/edge-pad/dtype-cast)
    import ml_dtypes

    d0 = np.asarray(data, dtype=np.float32)[:, :, 0, :]  # [100, 166, 768]
    pos = np.clip(np.arange(PPAD) - 5, 0, P - 1)
    dp = d0[:, pos, :]  # [100, 176, 768] with clip-pads baked in
    dp = np.transpose(dp, (0, 2, 1))  # [100, 768, 176]
    dp = dp.reshape(NSNIP, EC, 128, PPAD).transpose(0, 2, 1, 3)
    dataT2p = np.ascontiguousarray(
        dp.reshape(NSNIP * 128, EC * PPAD).astype(ml_dtypes.bfloat16)
    )
    wT = np.asarray(w, dtype=np.float32).T  # [768, 11]
    w2 = wT.reshape(EC, 128, W).transpose(1, 0, 2).reshape(128, EC * W)
    w2 = np.ascontiguousarray(w2.astype(ml_dtypes.bfloat16))
    return dataT2p, w2


def kernel(inputs, code_snippet_id, data, w, _trace=False):
    import ml_dtypes
    from concourse.bass_utils import run_bass_kernel_spmd

    nc = _get_nc()
    inputs = np.asarray(inputs, dtype=np.int32)
    code_snippet_id = np.asarray(code_snippet_id, dtype=np.int32)
    dataT2p, w2 = _prep_shared(data, w)

    in_maps = []
    for ci in range(N_CORES):
        b0 = ci * BPC
        in_maps.append(
            {
                "snips": np.ascontiguousarray(
                    code_snippet_id[b0 : b0 + BPC].reshape(1, BPC)
                ),
                "inps": np.ascontiguousarray(
                    inputs[b0 : b0 + BPC]
                    .reshape(1, BPC * S)
                    .astype(ml_dtypes.bfloat16)
                ),
                "dataT2p": dataT2p,
                "w2": w2,
            }
        )

    res = run_bass_kernel_spmd(
        nc, in_maps, core_ids=list(range(N_CORES)), trace=_trace
    )
    _cache["last_results"] = res
    out = np.concatenate(
        [res.results[i]["out"].reshape(BPC, S, E) for i in range(N_CORES)],
        axis=0,
    ).astype(np.float32)
    return out
